# revision 26
# baseline (speedup 1.0000x reference)
"""Trainium2 Bass kernel for MQA causal attention (16 q heads, 1 shared kv head).

Sharding: 2-way data-parallel over batch x 4-way tensor-parallel over query
heads: core c handles batch c//4 and query heads 4*(c%4)..4*(c%4)+3, with the
shared K/V head computed per core (replicated only within a batch group).
Each core emits a bf16 partial of its batch's out-projection; the host sums
the 4 partials per batch in f32.

v4 layout/scheduling notes (from trace analysis of v1-v3):
  - Matmul cost on HW is stream(N cols @2.4GHz) + serialized LDWEIGHTS
    (~110ns) unless consecutive matmuls share the stationary operand. All
    loops are ordered so every LDWEIGHTS serves >=2 matmuls: projections pair
    adjacent n-slices; attention interleaves the two head-pairs (shared krot/
    vnat chunk stationaries); out-projection streams 4 psy banks per attnT
    chunk.
  - Scores transposed: simT[keys, h*q] = kT.T @ qT. The 1/sqrt(d) scale rides
    the exp's free affine (activation scale=SCALE) so q/k rope tables are
    shared and unscaled.
  - exp batched per (head-pair, chunk-pair): one ACT instruction over a
    2-bank PSUM tile (N=1024). Causal masking via gpsimd affine_select on the
    two diagonal chunks only; the off-diagonal half of the last chunk is
    trimmed from the pss/psd/psa matmuls (queries 0:128 never see chunk 2t+1).
  - softmax denominator: ones-column matmuls accumulated in PSUM;
    reciprocal_approx_fast + gpsimd partition_broadcast + one DVE multiply
    normalizes attnT per qtile.
  - The out-projection of qtile t is deferred: emitted inside attention of
    qtile t+1 (or the next rep's first projection slice-pair) so the in-order
    PE queue always has ready work during exp waits; y staged bf16, DMA'd on
    the ACT queue (x loads ride the SP queue); psy->ysb casts alternate
    DVE/ACT.
  - PSUM budget (8 banks): pair tag (4KB) x2 bufs = 4 banks (proj slice-pair
    accumulators / score hp-tiles / outproj psy pairs), psa tag x2 = 2
    (attn@V accumulators per head-pair + v-transpose staging), den tag x2 = 2.
"""

import os
import sys
from contextlib import ExitStack

import numpy as np

for _p in ("/opt/trn_rl_repo",):
    if os.path.isdir(_p) and _p not in sys.path:
        sys.path.insert(0, _p)

import ml_dtypes

import concourse.bass as bass
import concourse.mybir as mybir
import concourse.tile as tile
from concourse import bacc
from concourse.bass_utils import run_bass_kernel_spmd
from concourse.masks import make_identity

HEADS = 16
D = 128
SCALE = D ** -0.5
N_CORES = 8
BSPLIT = 2                            # batch groups
HPC = HEADS // (N_CORES // BSPLIT)    # query heads per core (4)

F32 = mybir.dt.float32
BF16 = mybir.dt.bfloat16


def _rope(nc, sb_pool, ps, out_slice, cos_s, sin_s):
    """out_slice(bf16) = ps*cos_s + rot(ps)*sin_s via partition-offset DVE
    reads; sin_s arrives pre-signed from the host (rows 0-63 negated)."""
    L = ps.shape[-1]
    t1 = sb_pool.tile([128, L], F32, tag="ropet1")
    nc.vector.tensor_mul(t1, ps, cos_s)
    t2 = sb_pool.tile([128, L], F32, tag="ropet2")
    nc.vector.tensor_mul(t2[0:64, :], ps[64:128, :], sin_s[0:64, :])
    nc.vector.tensor_mul(t2[64:128, :], ps[0:64, :], sin_s[64:128, :])
    nc.vector.tensor_add(out_slice, t1, t2)


def build_nc(B, N, DIM, HL, reps=1):
    """One SPMD program: HPC query heads + shared kv head, one batch element.

    reps>1 repeats the whole computation (same output) for timing-by-
    difference: NEFF(reps=K) wall minus NEFF(reps=1) wall = (K-1) * body.
    """
    assert B == 2 and HL == 2 and N == 2048
    DC = DIM // 128           # contraction chunks for projections
    SL = 512                  # projection n-slice length
    NS = N // SL              # n slices
    KPS = SL // 128           # key chunks per slice
    NQT = N // 256            # 256-row query tiles
    NHP = HPC // 2            # head pairs (2)

    nc = bacc.Bacc(None, target_bir_lowering=False)
    xT = nc.declare_dram_parameter("xT", [DIM, N], BF16, isOutput=False)
    wq = nc.declare_dram_parameter("wq", [DIM, HPC * D], BF16, isOutput=False)
    wkv = nc.declare_dram_parameter("wkv", [DIM, 2 * D], BF16, isOutput=False)
    wout = nc.declare_dram_parameter("wout", [HPC * D, DIM], BF16, isOutput=False)
    cost = nc.declare_dram_parameter("cost", [D, N], BF16, isOutput=False)
    sint = nc.declare_dram_parameter("sint", [D, N], BF16, isOutput=False)
    y = nc.declare_dram_parameter("y", [N, DIM], BF16, isOutput=True)

    with ExitStack() as ctx:
        tc = ctx.enter_context(tile.TileContext(nc))
        consts = ctx.enter_context(tc.tile_pool(name="consts", bufs=1))
        xpool = ctx.enter_context(tc.tile_pool(name="xpool", bufs=3))
        proj = ctx.enter_context(tc.tile_pool(name="proj", bufs=1))
        sb = ctx.enter_context(tc.tile_pool(name="sb", bufs=2))
        outp = ctx.enter_context(tc.tile_pool(name="outp", bufs=3))
        psp = ctx.enter_context(tc.tile_pool(name="psp", bufs=2, space="PSUM"))

        ident = consts.tile([128, 128], BF16)
        make_identity(nc, ident)
        ones_col = consts.tile([128, 1], BF16)
        nc.vector.memset(ones_col, 1.0)

        wq_sb = consts.tile([128, DC, HPC * D], BF16)
        wkv_sb = consts.tile([128, DC, 2 * D], BF16)
        nc.sync.dma_start(
            wq_sb, wq.rearrange("(c p) m -> p c m", p=128))
        nc.sync.dma_start(
            wkv_sb, wkv.rearrange("(c p) m -> p c m", p=128))
        # bulk constants go on the ACT HWDGE queue so they don't delay the
        # x-tile stream on the SP queue
        wout_sb = consts.tile([128, HPC, DIM], BF16)
        nc.scalar.dma_start(wout_sb, wout.rearrange("(c p) m -> p c m", p=128))
        cos_sb = consts.tile([128, N], BF16)
        sin_sb = consts.tile([128, N], BF16)
        nc.scalar.dma_start(cos_sb, cost[:, :])
        nc.scalar.dma_start(sin_sb, sint[:, :])

        pending = []   # FIFO of (m, attnT list) out-proj chunks to emit

        def _outproj_m(m, aTs):
            ysb = outp.tile([128, DIM], BF16, tag="ysb")
            pairs = [psp.tile([128, 2, 512], F32, tag="pair", name="psy")
                     for _ in range(DIM // 1024)]
            nmm = 0
            for hp in range(NHP):
                for hc in range(2):
                    st = nmm == 0
                    sp = nmm == HPC - 1
                    nmm += 1
                    for pi, psy in enumerate(pairs):
                        for s in range(2):
                            nc.tensor.matmul(
                                psy[:, s],
                                aTs[hp][:, hc, m * 128:(m + 1) * 128],
                                wout_sb[:, 2 * hp + hc,
                                        (2 * pi + s) * 512:
                                        (2 * pi + s + 1) * 512],
                                start=st, stop=sp)
            for pi, psy in enumerate(pairs):
                eng = nc.vector.tensor_copy if pi == 0 else nc.scalar.copy
                eng(ysb[:, pi * 1024:(pi + 1) * 1024], psy)
            nc.scalar.dma_start(y[m * 128:(m + 1) * 128, :], ysb)

        def flush_one():
            if pending:
                _outproj_m(*pending.pop(0))

        for rep in range(reps):
            first = rep == 0
            qrot = [proj.tile([128, 2, N], BF16, tag=f"qrot{hp}",
                              name=f"qrot{hp}") for hp in range(NHP)]
            attnT = [proj.tile([128, 2, N], BF16, tag=f"attnT{hp}",
                               name=f"attnT{hp}") for hp in range(NHP)]
            krot = proj.tile([128, N], BF16, tag="krot")
            vnat = proj.tile([128, N // 128, D], BF16, tag="vnat")

            # ---- projections + rope, paired n-slices. Each weight chunk is
            # stationary for two consecutive matmuls (slice a / slice b) so
            # the next LDWEIGHTS hides under the current 512-col stream.
            for u in range(NS // 2):
                sls = [slice((2 * u + s) * SL, (2 * u + s + 1) * SL)
                       for s in range(2)]
                xts = []
                for s in range(2):
                    xt = xpool.tile([128, DC, SL], BF16, tag="xt", name="xt")
                    nc.sync.dma_start(
                        xt, xT.rearrange("(c p) n -> p c n", p=128)[:, :, sls[s]])
                    xts.append(xt)

                # v first: its psum->sbuf copy rides ACT
                psv = psp.tile([128, 2, SL], F32, tag="pair")
                for dc in range(DC):
                    for s in range(2):
                        nc.tensor.matmul(
                            psv[:, s], wkv_sb[:, dc, D:2 * D], xts[s][:, dc, :],
                            start=(dc == 0), stop=(dc == DC - 1))
                vt = sb.tile([128, 2, SL], BF16, tag="vt")
                nc.scalar.copy(vt, psv)
                for h in range(HPC):
                    psq = psp.tile([128, 2, SL], F32, tag="pair")
                    for dc in range(DC):
                        for s in range(2):
                            nc.tensor.matmul(
                                psq[:, s], wq_sb[:, dc, h * D:(h + 1) * D],
                                xts[s][:, dc, :],
                                start=(dc == 0), stop=(dc == DC - 1))
                    for s in range(2):
                        _rope(nc, sb, psq[:, s], qrot[h // 2][:, h % 2, sls[s]],
                              cos_sb[:, sls[s]], sin_sb[:, sls[s]])
                psk = psp.tile([128, 2, SL], F32, tag="pair")
                for dc in range(DC):
                    for s in range(2):
                        nc.tensor.matmul(
                            psk[:, s], wkv_sb[:, dc, 0:D], xts[s][:, dc, :],
                            start=(dc == 0), stop=(dc == DC - 1))
                for s in range(2):
                    _rope(nc, sb, psk[:, s], krot[:, sls[s]],
                          cos_sb[:, sls[s]], sin_sb[:, sls[s]])
                # v transposes last: vt's ACT copy lands during the q/k mms
                for s in range(2):
                    for kc in range(KPS):
                        pst = psp.tile([128, 128], BF16, tag="psa")
                        nc.tensor.transpose(
                            pst, vt[:, s, kc * 128:(kc + 1) * 128], ident)
                        nc.vector.tensor_copy(
                            vnat[:, (2 * u + s) * KPS + kc, :], pst)

            # ---- attention, head-pairs interleaved per qtile ----
            def _attn_tail(t, jp, first, last, psas, exs):
                """psa accumulating matmuls for chunk pair jp of qtile t
                (chunks 2*jp, 2*jp+1), both head pairs. The off-diagonal half
                of the diagonal chunk (queries 0:128 vs keys 2t*128+128+) is
                trimmed. The diagonal pair is processed FIRST (its full-width
                s=0 matmul opens the accumulation group) so its exp+select
                chain hides under the plain chunk pairs."""
                for s in range(2):
                    cut = 128 if (jp == t and s == 1) else 0
                    st = first and s == 0
                    sp = last and s == 1
                    for hp in range(NHP):
                        nc.tensor.matmul(
                            psas[hp][:, :, cut:256], vnat[:, 2 * jp + s, :],
                            exs[hp][:, s, :, cut:256],
                            start=st, stop=sp, skip_group_check=True)

            pend = None
            for t in range(NQT):
                psas = [psp.tile([128, 2, 256], F32, tag="psa", name="psa")
                        for _ in range(NHP)]
                accs = [sb.tile([128, 2, 256], BF16, tag="acc", name="acc")
                        for _ in range(NHP)]
                qsls = [qrot[hp][:, :, t * 256:(t + 1) * 256]
                        for hp in range(NHP)]
                jporder = [t] + list(range(t))
                for idx, jp in enumerate(jporder):
                    pps = [psp.tile([128, 2, 2, 256], F32, tag="pair",
                                    name="pp") for _ in range(NHP)]
                    for s in range(2):
                        cut = 128 if (jp == t and s == 1) else 0
                        for hp in range(NHP):
                            nc.tensor.matmul(
                                pps[hp][:, s, :, cut:256],
                                krot[:, (2 * jp + s) * 128:
                                     (2 * jp + s + 1) * 128],
                                qsls[hp][:, :, cut:256],
                                start=True, stop=True)
                    if pend is not None:
                        _attn_tail(*pend)
                        pend = None
                    exs = [sb.tile([128, 2, 2, 256], BF16, tag="exp",
                                   bufs=4, name="ex") for _ in range(NHP)]
                    for hp in range(NHP):
                        nc.scalar.activation(
                            exs[hp], pps[hp],
                            mybir.ActivationFunctionType.Exp, scale=SCALE)
                    if jp == t:
                        # diagonal chunks: keep where q - p - base >= 0
                        for hp in range(NHP):
                            nc.gpsimd.affine_select(
                                out=exs[hp][:, 0], in_=exs[hp][:, 0],
                                compare_op=mybir.AluOpType.is_ge, fill=0.0,
                                base=0, pattern=[[0, 2], [1, 256]],
                                channel_multiplier=-1)
                            nc.gpsimd.affine_select(
                                out=exs[hp][:, 1, :, 128:256],
                                in_=exs[hp][:, 1, :, 128:256],
                                compare_op=mybir.AluOpType.is_ge, fill=0.0,
                                base=0, pattern=[[0, 2], [1, 128]],
                                channel_multiplier=-1)
                    # denominator: DVE accumulates exp sums (bf16, 2x rate);
                    # diagonal (first) pair initializes the accumulator
                    for hp in range(NHP):
                        if idx == 0:
                            nc.vector.tensor_copy(
                                accs[hp][:, :, 0:128], exs[hp][:, 0, :, 0:128])
                            nc.vector.tensor_add(
                                accs[hp][:, :, 128:256],
                                exs[hp][:, 0, :, 128:256],
                                exs[hp][:, 1, :, 128:256])
                        else:
                            nc.vector.tensor_add(
                                accs[hp], accs[hp], exs[hp][:, 0])
                            nc.vector.tensor_add(
                                accs[hp], accs[hp], exs[hp][:, 1])
                    pend = (t, jp, idx == 0, idx == t, psas, exs)
                # fill the exp-wait of the last pair with half the pending
                # out-projection, finish this qtile, then emit the other half
                # to cover the next qtile's psa-rotation wait
                flush_one()
                flush_one()
                _attn_tail(*pend)
                pend = None
                psdns = [psp.tile([1, 2, 256], F32, tag="den", name="psdn")
                         for _ in range(NHP)]
                for hp in range(NHP):
                    nc.tensor.matmul(psdns[hp], ones_col, accs[hp],
                                     start=True, stop=True)
                for hp in range(NHP):
                    den = sb.tile([1, 2, 256], F32, tag="den", name="den")
                    nc.vector.reciprocal_approx_fast(den, psdns[hp])
                    bc = sb.tile([128, 2, 256], F32, tag="bc", name="bc")
                    nc.gpsimd.partition_broadcast(bc, den)
                    nc.vector.tensor_mul(
                        attnT[hp][:, :, t * 256:(t + 1) * 256], psas[hp], bc)
                pending.append((2 * t, attnT))
                pending.append((2 * t + 1, attnT))
            if rep == reps - 1:
                while pending:
                    flush_one()

    nc.finalize()
    return nc


def make_host_inputs(x, Wq, Wkv, Wout, HL):
    """Shard + precompute per-core input maps (host side)."""
    B, N, DIM = x.shape
    bf = ml_dtypes.bfloat16
    xT = [np.ascontiguousarray(x[b].T).astype(bf) for b in range(B)]
    inv = 1.0 / (10000.0 ** (np.arange(0, D, 2, dtype=np.float64) / D))
    fr = np.arange(N, dtype=np.float64)[:, None] * inv[None, :]
    pos = np.concatenate([fr, fr], axis=-1)              # [N, D]
    cos_t = np.cos(pos).T.astype(np.float32)             # [D, N]
    sin_t = np.sin(pos).T.astype(np.float32)
    sign = np.ones((D, 1), np.float32)
    sign[:D // 2] = -1.0
    sin_r = sin_t * sign            # fold rotate_half's sign into the table
    shared = dict(
        wkv=Wkv.astype(bf),
        cost=np.ascontiguousarray(cos_t).astype(bf),
        sint=np.ascontiguousarray(sin_r).astype(bf))
    in_maps = []
    groups = N_CORES // BSPLIT
    for c in range(N_CORES):
        b, hg = c // groups, c % groups
        lo, hi = hg * HPC * D, (hg + 1) * HPC * D
        in_maps.append(dict(
            shared, xT=xT[b],
            wq=np.ascontiguousarray(Wq[:, lo:hi]).astype(bf),
            wout=np.ascontiguousarray(Wout[lo:hi, :]).astype(bf)))
    return in_maps


def kernel(x, Wq, Wkv, Wout):
    B, N, DIM = x.shape
    HL = HEADS // N_CORES
    nc = build_nc(B, N, DIM, HL)
    in_maps = make_host_inputs(x, Wq, Wkv, Wout, HL)
    res = run_bass_kernel_spmd(nc, in_maps, core_ids=list(range(N_CORES)))
    y = np.zeros((B, N, DIM), np.float32)
    groups = N_CORES // BSPLIT
    for c, r in enumerate(res.results):
        y[c // groups] += r["y"].astype(np.float32)
    return y
